# revision 2
# baseline (speedup 1.0000x reference)
"""Trainium2 Bass kernel: CrossAttentionBlock, data-parallel over batch on 8 NeuronCores.

Per-core computation (one batch element b):
    Q = query[b] @ Wq.T + bq          [1024, 512]
    K = key[b]   @ Wk.T + bk          [2048, 512]
    V = key[b]   @ Wv.T + bv          [2048, 512]
    S = Q @ K.T / sqrt(512)           [1024, 2048]
    out = softmax(S, axis=-1) @ V     [1024, 512]

Device-side layout strategy (everything stays in "transposed world" so the
TensorEngine contraction dim is always on SBUF partitions, no on-device
transposes needed):
  - host passes query[b].T ("qT" [512,1024]) and key[b].T ("kT" [512,2048]),
    plus transposed weights WqT/WkT/WvT = W.T ([d_in, d_out]).
  - Q^T[e,i]  = sum_d WqT[d,e] qT[d,i]     (PE, accumulate over 4 d-subtiles)
  - K^T[e,k]  = sum_d WkT[d,e] kT[d,k]
  - V[k,e]    = sum_d kT[d,k] WvT[d,e]
  - S^T[k,i]  = sum_e KT[e,k] QT[e,i]      (per 128-row k-tile, 512-col i-chunk)
  - E = exp(S^T * scale)                   (ScalarE, softmax w/o max-subtraction:
                                            scores ~ N(0,1), no overflow risk)
  - sumexp[:,i] = ones[128,128].T @ E      (accumulated over k-tiles; every
                                            partition gets the same row sums)
  - att^T[e,i] = sum_k V[k,e] E[k,i]       (accumulated over k-tiles)
  - out^T = att^T * (1/sumexp)             (VectorE reciprocal + multiply)
Host transposes out^T back to [1024, 512] per batch element.
"""

import numpy as np

import concourse.bass as bass
import concourse.mybir as mybir
import concourse.tile as tile
from concourse import bacc
from concourse.bass_utils import run_bass_kernel_spmd

P = 128
D_MODEL = 512
DT = D_MODEL // P      # contraction subtiles (4)
ET = D_MODEL // P      # model-dim output tiles (4)
LQ = 1024
LK = 2048
NKT = LK // P          # key tiles (16)
F = 512                # matmul free dim / query-chunk width
NIC = LQ // F          # query chunks (2)
NKC = LK // F          # key chunks for K^T projection (4)
N_CORES = 8
SCALE = float(D_MODEL) ** -0.5

f32 = mybir.dt.float32
f32r = mybir.dt.float32r
AF = mybir.ActivationFunctionType

# "f32r": single-pass fp32 matmuls (full PE rate, slightly reduced precision)
# "f32" : true fp32 matmuls (4x slower, full precision)
MM_DTYPE = "f32r"
MMD = f32r if MM_DTYPE == "f32r" else f32


def _mm(ap):
    return ap


def build_nc():
    # Bacc: its compile() pass splits multi-sem waits into EventSemaphores
    # (walrus allows only ONE sync wait per engine instruction).
    nc = bacc.Bacc()
    qT = nc.declare_dram_parameter("qT", [D_MODEL, LQ], MMD, isOutput=False)
    kT = nc.declare_dram_parameter("kT", [D_MODEL, LK], MMD, isOutput=False)
    wqT = nc.declare_dram_parameter("wqT", [D_MODEL, D_MODEL], MMD, isOutput=False)
    wkT = nc.declare_dram_parameter("wkT", [D_MODEL, D_MODEL], MMD, isOutput=False)
    wvT = nc.declare_dram_parameter("wvT", [D_MODEL, D_MODEL], MMD, isOutput=False)
    bqT = nc.declare_dram_parameter("bqT", [P, ET], f32, isOutput=False)
    bkT = nc.declare_dram_parameter("bkT", [P, ET], f32, isOutput=False)
    bvB = nc.declare_dram_parameter("bvB", [P, D_MODEL], f32, isOutput=False)
    ones = nc.declare_dram_parameter("ones", [P, P], MMD, isOutput=False)
    outT = nc.declare_dram_parameter("outT", [D_MODEL, LQ], f32, isOutput=True)

    qT_r = qT.rearrange("(dt p) i -> p dt i", p=P)
    kT_r = kT.rearrange("(dt p) k -> p dt k", p=P)
    wq_r = wqT.rearrange("(dt p) e -> p dt e", p=P)
    wk_r = wkT.rearrange("(dt p) e -> p dt e", p=P)
    wv_r = wvT.rearrange("(dt p) e -> p dt e", p=P)
    outT_r = outT.rearrange("(et p) i -> p et i", p=P)

    with (
        tile.TileContext(nc) as tc,
        tc.tile_pool(name="big", bufs=1) as big,
        tc.tile_pool(name="work", bufs=3) as work,
        tc.tile_pool(name="mmp", bufs=3, space="PSUM") as mmp,
        tc.tile_pool(name="attp", bufs=4, space="PSUM") as attp,
        tc.tile_pool(name="sump", bufs=1, space="PSUM") as sump,
    ):
        qT_sb = big.tile([P, DT, LQ], MMD, tag="qT")
        kT_sb = big.tile([P, DT, LK], MMD, tag="kT")
        wq_sb = big.tile([P, DT, D_MODEL], MMD, tag="wq")
        wk_sb = big.tile([P, DT, D_MODEL], MMD, tag="wk")
        wv_sb = big.tile([P, DT, D_MODEL], MMD, tag="wv")
        bq_sb = big.tile([P, ET], f32, tag="bq")
        bk_sb = big.tile([P, ET], f32, tag="bk")
        bv_sb = big.tile([P, D_MODEL], f32, tag="bv")
        QT_sb = big.tile([P, ET, LQ], MMD, tag="QT")
        KT_sb = big.tile([P, ET, LK], MMD, tag="KT")
        V_sb = big.tile([P, NKT, D_MODEL], MMD, tag="V")
        out_sb = big.tile([P, ET, LQ], f32, tag="out")
        ones_sb = big.tile([P, P], MMD, tag="ones")

        # ---- input DMAs (ordered to match first consumption) ----
        def dma_k_chunk(kc):
            sl = slice(kc * F, (kc + 1) * F)
            nc.sync.dma_start(kT_sb[:, :, sl], kT_r[:, :, sl])

        def dma_q_chunk(ic):
            sl = slice(ic * F, (ic + 1) * F)
            nc.sync.dma_start(qT_sb[:, :, sl], qT_r[:, :, sl])

        nc.sync.dma_start(ones_sb[:], ones[:])
        nc.sync.dma_start(wq_sb[:], wq_r)
        nc.sync.dma_start(bq_sb[:], bqT[:])
        dma_q_chunk(0)
        nc.sync.dma_start(wk_sb[:], wk_r)
        nc.sync.dma_start(bk_sb[:], bkT[:])
        nc.sync.dma_start(wv_sb[:], wv_r)
        nc.sync.dma_start(bv_sb[:], bvB[:])
        dma_k_chunk(0)
        dma_q_chunk(1)
        dma_k_chunk(1)
        dma_k_chunk(2)
        dma_k_chunk(3)

        # ---- projections, interleaved by DMA-chunk arrival so the PE
        # fills kT-transfer wait time with already-available work ----
        def q_proj(ic):
            isl = slice(ic * F, (ic + 1) * F)
            for et in range(ET):
                ps = mmp.tile([P, F], f32, tag="mm", name=f"ps_q{ic}{et}")
                for d in range(DT):
                    nc.tensor.matmul(
                        ps[:],
                        wq_sb[:, d, et * P:(et + 1) * P],
                        qT_sb[:, d, isl],
                        start=(d == 0),
                        stop=(d == DT - 1),
                    )
                nc.scalar.activation(
                    QT_sb[:, et, isl], ps[:], AF.Identity, bias=bq_sb[:, et:et + 1]
                )

        def k_proj(kc):
            ksl = slice(kc * F, (kc + 1) * F)
            for et in range(ET):
                ps = mmp.tile([P, F], f32, tag="mm", name=f"ps_k{kc}{et}")
                for d in range(DT):
                    nc.tensor.matmul(
                        ps[:],
                        wk_sb[:, d, et * P:(et + 1) * P],
                        kT_sb[:, d, ksl],
                        start=(d == 0),
                        stop=(d == DT - 1),
                    )
                nc.scalar.activation(
                    KT_sb[:, et, ksl], ps[:], AF.Identity, bias=bk_sb[:, et:et + 1]
                )

        def v_proj(kc):
            for kt in range(4 * kc, 4 * kc + 4):
                ps = mmp.tile([P, F], f32, tag="mm", name=f"ps_v{kt}")
                for d in range(DT):
                    nc.tensor.matmul(
                        ps[:],
                        kT_sb[:, d, kt * P:(kt + 1) * P],
                        wv_sb[:, d, :],
                        start=(d == 0),
                        stop=(d == DT - 1),
                    )
                nc.vector.tensor_add(V_sb[:, kt, :], ps[:], bv_sb[:])

        q_proj(0)
        k_proj(0)
        v_proj(0)
        q_proj(1)
        k_proj(1)
        v_proj(1)
        k_proj(2)
        v_proj(2)
        k_proj(3)
        v_proj(3)

        # ---- attention ----
        for ic in range(NIC):
            isl = slice(ic * F, (ic + 1) * F)
            att = [
                attp.tile([P, F], f32, tag="att", name=f"att_{ic}_{j}")
                for j in range(ET)
            ]
            sum_ps = sump.tile([P, F], f32, tag="sum")

            def s_tile(kt, isl=isl):
                ps = mmp.tile([P, F], f32, tag="mm")
                for et in range(ET):
                    nc.tensor.matmul(
                        ps[:],
                        _mm(KT_sb[:, et, kt * P:(kt + 1) * P]),
                        _mm(QT_sb[:, et, isl]),
                        start=(et == 0),
                        stop=(et == ET - 1),
                    )
                return ps

            # software-pipelined: S(kt+1) on PE overlaps exp(kt) on ScalarE
            s_prev = s_tile(0)
            for kt in range(NKT):
                s_next = s_tile(kt + 1) if kt + 1 < NKT else None
                E = work.tile([P, F], MMD, tag="E")
                nc.scalar.activation(E[:], s_prev[:], AF.Exp, scale=SCALE)
                nc.tensor.matmul(
                    sum_ps[:], _mm(ones_sb[:]), _mm(E[:]),
                    start=(kt == 0), stop=(kt == NKT - 1),
                )
                for et in range(ET):
                    nc.tensor.matmul(
                        att[et][:],
                        _mm(V_sb[:, kt, et * P:(et + 1) * P]),
                        _mm(E[:]),
                        start=(kt == 0),
                        stop=(kt == NKT - 1),
                    )
                s_prev = s_next

            recip = work.tile([P, F], f32, tag="recip")
            nc.vector.reciprocal(recip[:], sum_ps[:])
            for et in range(ET):
                nc.vector.tensor_mul(out_sb[:, et, isl], att[et][:], recip[:])
                nc.sync.dma_start(outT_r[:, et, isl], out_sb[:, et, isl])

    nc.finalize()
    return nc


_NC_CACHE = None


def _get_nc():
    global _NC_CACHE
    if _NC_CACHE is None:
        _NC_CACHE = build_nc()
    return _NC_CACHE


def _prep_in_maps(query, key, Wq, bq, Wk, bk, Wv, bv):
    c = np.ascontiguousarray
    shared = {
        "wqT": c(Wq.T),
        "wkT": c(Wk.T),
        "wvT": c(Wv.T),
        "bqT": c(bq.reshape(ET, P).T),
        "bkT": c(bk.reshape(ET, P).T),
        "bvB": c(np.broadcast_to(bv, (P, D_MODEL))),
        "ones": np.ones((P, P), np.float32),
    }
    return [
        {"qT": c(query[b].T), "kT": c(key[b].T), **shared}
        for b in range(N_CORES)
    ]


def kernel(**inputs):
    query = np.asarray(inputs["query"], np.float32)
    key = np.asarray(inputs["key"], np.float32)
    Wq = np.asarray(inputs["Wq"], np.float32)
    bq = np.asarray(inputs["bq"], np.float32)
    Wk = np.asarray(inputs["Wk"], np.float32)
    bk = np.asarray(inputs["bk"], np.float32)
    Wv = np.asarray(inputs["Wv"], np.float32)
    bv = np.asarray(inputs["bv"], np.float32)

    in_maps = _prep_in_maps(query, key, Wq, bq, Wk, bk, Wv, bv)
    res = run_bass_kernel_spmd(_get_nc(), in_maps, list(range(N_CORES)))
    global LAST_RES
    LAST_RES = res
    out = np.stack([res.results[b]["outT"].T for b in range(N_CORES)])
    return np.ascontiguousarray(out.astype(np.float32))


LAST_RES = None



# revision 4
# speedup vs baseline: 1.1205x; 1.1205x over previous
"""Trainium2 Bass kernel: CrossAttentionBlock, data-parallel over batch on 8 NeuronCores.

Per-core computation (one batch element b):
    Q = query[b] @ Wq.T + bq          [1024, 512]
    K = key[b]   @ Wk.T + bk          [2048, 512]
    V = key[b]   @ Wv.T + bv          [2048, 512]
    S = Q @ K.T / sqrt(512)           [1024, 2048]
    out = softmax(S, axis=-1) @ V     [1024, 512]

Device-side layout strategy (everything stays in "transposed world" so the
TensorEngine contraction dim is always on SBUF partitions, no on-device
transposes needed):
  - host passes query[b].T ("qT" [512,1024]) and key[b].T ("kT" [512,2048]),
    plus transposed weights WqT/WkT/WvT = W.T ([d_in, d_out]), all fp16
    (fp16 matmuls run at the same PE rate as fp32r but halve DMA/SBUF and
    enable fast weight loads; end-to-end rel err ~1e-3 vs 2e-2 budget).
  - Q^T[e,i]  = sum_d WqT[d,e] qT[d,i]     (PE, accumulate over 4 d-subtiles)
  - K^T[e,k]  = sum_d WkT[d,e] kT[d,k]
  - V[k,e]    = sum_d kT[d,k] WvT[d,e]
  - S^T[k,i]  = sum_e KT[e,k] QT[e,i]      (per 128-row k-tile, 512-col i-chunk)
  - E = exp(S^T * scale)                   (ScalarE, softmax w/o max-subtraction:
                                            E max ~4e3 fits fp16 range)
  - sumexp[:,i] = ones[128,128].T @ E      (accumulated over k-tiles; every
                                            partition gets the same row sums)
  - att^T[e,i] = sum_k V[k,e] E[k,i]       (accumulated over k-tiles)
  - out^T = att^T * (1/sumexp)             (ScalarE reciprocal + VectorE multiply)
Host transposes out^T back to [1024, 512] per batch element.
"""

import numpy as np

import concourse.bass as bass
import concourse.mybir as mybir
import concourse.tile as tile
from concourse import bacc
from concourse.bass_utils import run_bass_kernel_spmd

P = 128
D_MODEL = 512
DT = D_MODEL // P      # contraction subtiles (4)
ET = D_MODEL // P      # model-dim output tiles (4)
LQ = 1024
LK = 2048
NKT = LK // P          # key tiles (16)
F = 512                # matmul free dim / query-chunk width
NIC = LQ // F          # query chunks (2)
NKC = LK // F          # key chunks for K^T projection (4)
N_CORES = 8
SCALE = float(D_MODEL) ** -0.5

f32 = mybir.dt.float32
f16 = mybir.dt.float16
AF = mybir.ActivationFunctionType

MMD = f16              # matmul operand dtype


def build_nc():
    # Bacc: its compile() pass splits multi-sem waits into EventSemaphores
    # (walrus allows only ONE sync wait per engine instruction).
    nc = bacc.Bacc()
    qT = nc.declare_dram_parameter("qT", [D_MODEL, LQ], MMD, isOutput=False)
    kT = nc.declare_dram_parameter("kT", [D_MODEL, LK], MMD, isOutput=False)
    wqT = nc.declare_dram_parameter("wqT", [D_MODEL, D_MODEL], MMD, isOutput=False)
    wkT = nc.declare_dram_parameter("wkT", [D_MODEL, D_MODEL], MMD, isOutput=False)
    wvT = nc.declare_dram_parameter("wvT", [D_MODEL, D_MODEL], MMD, isOutput=False)
    bqT = nc.declare_dram_parameter("bqT", [P, ET], f32, isOutput=False)
    bkT = nc.declare_dram_parameter("bkT", [P, ET], f32, isOutput=False)
    bvB = nc.declare_dram_parameter("bvB", [P, D_MODEL], f32, isOutput=False)
    ones = nc.declare_dram_parameter("ones", [P, P], MMD, isOutput=False)
    outT = nc.declare_dram_parameter("outT", [D_MODEL, LQ], f16, isOutput=True)

    qT_r = qT.rearrange("(dt p) i -> p dt i", p=P)
    kT_r = kT.rearrange("(dt p) k -> p dt k", p=P)
    wq_r = wqT.rearrange("(dt p) e -> p dt e", p=P)
    wk_r = wkT.rearrange("(dt p) e -> p dt e", p=P)
    wv_r = wvT.rearrange("(dt p) e -> p dt e", p=P)
    outT_r = outT.rearrange("(et p) i -> p et i", p=P)

    with (
        tile.TileContext(nc) as tc,
        tc.tile_pool(name="big", bufs=1) as big,
        tc.tile_pool(name="work", bufs=3) as work,
        tc.tile_pool(name="mmp", bufs=3, space="PSUM") as mmp,
        tc.tile_pool(name="attp", bufs=4, space="PSUM") as attp,
        tc.tile_pool(name="sump", bufs=1, space="PSUM") as sump,
    ):
        qT_sb = big.tile([P, DT, LQ], MMD, tag="qT")
        kT_sb = big.tile([P, DT, LK], MMD, tag="kT")
        wq_sb = big.tile([P, DT, D_MODEL], MMD, tag="wq")
        wk_sb = big.tile([P, DT, D_MODEL], MMD, tag="wk")
        wv_sb = big.tile([P, DT, D_MODEL], MMD, tag="wv")
        bq_sb = big.tile([P, ET], f32, tag="bq")
        bk_sb = big.tile([P, ET], f32, tag="bk")
        bv_sb = big.tile([P, D_MODEL], f32, tag="bv")
        QT_sb = big.tile([P, ET, LQ], MMD, tag="QT")
        KT_sb = big.tile([P, ET, LK], MMD, tag="KT")
        V_sb = big.tile([P, NKT, D_MODEL], MMD, tag="V")
        out_sb = big.tile([P, ET, LQ], f16, tag="out")
        ones_sb = big.tile([P, P], MMD, tag="ones")

        # ---- input DMAs, ordered so the first matmuls' deps land first ----
        def dma_k_chunk(kc):
            sl = slice(kc * F, (kc + 1) * F)
            nc.sync.dma_start(kT_sb[:, :, sl], kT_r[:, :, sl])

        def dma_q_chunk(ic):
            sl = slice(ic * F, (ic + 1) * F)
            nc.sync.dma_start(qT_sb[:, :, sl], qT_r[:, :, sl])

        nc.sync.dma_start(wq_sb[:], wq_r)
        nc.sync.dma_start(bq_sb[:], bqT[:])
        dma_q_chunk(0)
        nc.sync.dma_start(wk_sb[:], wk_r)
        nc.sync.dma_start(bk_sb[:], bkT[:])
        dma_k_chunk(0)
        nc.sync.dma_start(wv_sb[:], wv_r)
        nc.sync.dma_start(bv_sb[:], bvB[:])
        nc.sync.dma_start(ones_sb[:], ones[:])
        dma_q_chunk(1)
        dma_k_chunk(1)
        dma_k_chunk(2)
        dma_k_chunk(3)

        # ---- projections, interleaved by DMA-chunk arrival so the PE
        # fills kT-transfer wait time with already-available work ----
        def q_proj(ic):
            isl = slice(ic * F, (ic + 1) * F)
            for et in range(ET):
                ps = mmp.tile([P, F], f32, tag="mm", name=f"ps_q{ic}{et}")
                for d in range(DT):
                    nc.tensor.matmul(
                        ps[:],
                        wq_sb[:, d, et * P:(et + 1) * P],
                        qT_sb[:, d, isl],
                        start=(d == 0),
                        stop=(d == DT - 1),
                    )
                nc.scalar.activation(
                    QT_sb[:, et, isl], ps[:], AF.Identity, bias=bq_sb[:, et:et + 1]
                )

        def k_proj(kc):
            ksl = slice(kc * F, (kc + 1) * F)
            for et in range(ET):
                ps = mmp.tile([P, F], f32, tag="mm", name=f"ps_k{kc}{et}")
                for d in range(DT):
                    nc.tensor.matmul(
                        ps[:],
                        wk_sb[:, d, et * P:(et + 1) * P],
                        kT_sb[:, d, ksl],
                        start=(d == 0),
                        stop=(d == DT - 1),
                    )
                nc.scalar.activation(
                    KT_sb[:, et, ksl], ps[:], AF.Identity, bias=bk_sb[:, et:et + 1]
                )

        def v_proj(kc):
            for kt in range(4 * kc, 4 * kc + 4):
                ps = mmp.tile([P, F], f32, tag="mm", name=f"ps_v{kt}")
                for d in range(DT):
                    nc.tensor.matmul(
                        ps[:],
                        kT_sb[:, d, kt * P:(kt + 1) * P],
                        wv_sb[:, d, :],
                        start=(d == 0),
                        stop=(d == DT - 1),
                    )
                nc.vector.tensor_add(V_sb[:, kt, :], ps[:], bv_sb[:])

        q_proj(0)
        k_proj(0)
        v_proj(0)
        q_proj(1)
        k_proj(1)
        v_proj(1)
        k_proj(2)
        v_proj(2)
        k_proj(3)
        v_proj(3)

        # ---- attention ----
        for ic in range(NIC):
            isl = slice(ic * F, (ic + 1) * F)
            att = [
                attp.tile([P, F], f32, tag="att", name=f"att_{ic}_{j}")
                for j in range(ET)
            ]
            sum_ps = sump.tile([P, F], f32, tag="sum")

            def s_tile(kt, isl=isl):
                ps = mmp.tile([P, F], f32, tag="mm")
                for et in range(ET):
                    nc.tensor.matmul(
                        ps[:],
                        KT_sb[:, et, kt * P:(kt + 1) * P],
                        QT_sb[:, et, isl],
                        start=(et == 0),
                        stop=(et == ET - 1),
                    )
                return ps

            # software-pipelined: S(kt+1) on PE overlaps exp(kt) on ScalarE
            s_prev = s_tile(0)
            for kt in range(NKT):
                s_next = s_tile(kt + 1) if kt + 1 < NKT else None
                E = work.tile([P, F], MMD, tag="E")
                nc.scalar.activation(E[:], s_prev[:], AF.Exp, scale=SCALE)
                nc.tensor.matmul(
                    sum_ps[:], ones_sb[:], E[:],
                    start=(kt == 0), stop=(kt == NKT - 1),
                )
                for et in range(ET):
                    nc.tensor.matmul(
                        att[et][:],
                        V_sb[:, kt, et * P:(et + 1) * P],
                        E[:],
                        start=(kt == 0),
                        stop=(kt == NKT - 1),
                    )
                s_prev = s_next

            # 1/sumexp = Exp(-Ln(s)) on ScalarE: 2 x ~0.7us vs 3.4us for the
            # DVE RECIPROCAL (and ScalarE's Reciprocal LUT is blocked in bass)
            lnsum = work.tile([P, F], f32, tag="lnsum")
            nc.scalar.activation(lnsum[:], sum_ps[:], AF.Ln)
            recip = work.tile([P, F], f32, tag="recip")
            nc.scalar.activation(recip[:], lnsum[:], AF.Exp, scale=-1.0)
            for eh in range(2):  # two half-DMAs so transfer overlaps the muls
                for et in (2 * eh, 2 * eh + 1):
                    nc.vector.tensor_mul(out_sb[:, et, isl], att[et][:], recip[:])
                nc.sync.dma_start(
                    outT_r[:, 2 * eh:2 * eh + 2, isl],
                    out_sb[:, 2 * eh:2 * eh + 2, isl],
                )

    nc.finalize()
    return nc


_NC_CACHE = None


def _get_nc():
    global _NC_CACHE
    if _NC_CACHE is None:
        _NC_CACHE = build_nc()
    return _NC_CACHE


def _prep_in_maps(query, key, Wq, bq, Wk, bk, Wv, bv):
    c = np.ascontiguousarray
    h = np.float16
    shared = {
        "wqT": c(Wq.T.astype(h)),
        "wkT": c(Wk.T.astype(h)),
        "wvT": c(Wv.T.astype(h)),
        "bqT": c(bq.reshape(ET, P).T),
        "bkT": c(bk.reshape(ET, P).T),
        "bvB": c(np.broadcast_to(bv, (P, D_MODEL))),
        "ones": np.ones((P, P), h),
    }
    return [
        {"qT": c(query[b].T.astype(h)), "kT": c(key[b].T.astype(h)), **shared}
        for b in range(N_CORES)
    ]


def kernel(**inputs):
    query = np.asarray(inputs["query"], np.float32)
    key = np.asarray(inputs["key"], np.float32)
    Wq = np.asarray(inputs["Wq"], np.float32)
    bq = np.asarray(inputs["bq"], np.float32)
    Wk = np.asarray(inputs["Wk"], np.float32)
    bk = np.asarray(inputs["bk"], np.float32)
    Wv = np.asarray(inputs["Wv"], np.float32)
    bv = np.asarray(inputs["bv"], np.float32)

    in_maps = _prep_in_maps(query, key, Wq, bq, Wk, bk, Wv, bv)
    res = run_bass_kernel_spmd(_get_nc(), in_maps, list(range(N_CORES)))
    global LAST_RES
    LAST_RES = res
    out = np.stack(
        [res.results[b]["outT"].astype(np.float32).T for b in range(N_CORES)]
    )
    return np.ascontiguousarray(out)


LAST_RES = None


# revision 6
# speedup vs baseline: 1.2062x; 1.0764x over previous
"""Trainium2 Bass kernel: CrossAttentionBlock, data-parallel over batch on 8 NeuronCores.

Per-core computation (one batch element b):
    Q = query[b] @ Wq.T + bq          [1024, 512]
    K = key[b]   @ Wk.T + bk          [2048, 512]
    V = key[b]   @ Wv.T + bv          [2048, 512]
    S = Q @ K.T / sqrt(512)           [1024, 2048]
    out = softmax(S, axis=-1) @ V     [1024, 512]

Device-side layout ("transposed world": the TensorEngine contraction dim is
always on SBUF partitions, no on-device transposes needed):
  - host passes query[b].T ("qT" [512,1024]), key[b].T ("kT" [512,2048]) and
    transposed weights WqT/WkT/WvT ([d_in, d_out]), all fp16 (same PE rate as
    fp32r, half the DMA/SBUF traffic; end-to-end rel err ~1e-3 vs 2e-2 budget).
  - Q^T[e,i]  = sum_d WqT[d,e] qT[d,i]     (PE, accumulate over 4 d-subtiles)
  - K^T[e,k]  = sum_d WkT[d,e] kT[d,k]
  - V[k,e]    = sum_d kT[d,k] WvT[d,e]
  - S^T[k,i]  = sum_e KT[e,k] QT[e,i]      (per 128-row k-tile, 512-col i-chunk)
  - E = exp(S^T * scale)                   (ScalarE; no max-subtraction needed:
                                            E max ~4e3 fits fp16 range)
  - sumE[p,i] += E[p,i] per k-tile         (VectorE accumulate, fp32)
  - sum_ps = ones.T @ sumE                 (one PE pass: all-partition row sums)
  - att^T[e,i] = sum_k V[k,e] E[k,i]       (PE, accumulated over k-tiles)
  - out^T = att^T * recip(sumexp)          (DVE reciprocal_approx_fast + mul)
Host transposes out^T back to [1024, 512] per batch element.
"""

import numpy as np

import concourse.bass as bass
import concourse.mybir as mybir
import concourse.tile as tile
from concourse import bacc
from concourse.bass_utils import run_bass_kernel_spmd

P = 128
D_MODEL = 512
DT = D_MODEL // P      # contraction subtiles (4)
ET = D_MODEL // P      # model-dim output tiles (4)
LQ = 1024
LK = 2048
NKT = LK // P          # key tiles (16)
F = 512                # matmul free dim / query-chunk width
NIC = LQ // F          # query chunks (2)
NKC = LK // F          # key chunks for K^T projection (4)
N_CORES = 8
SCALE = float(D_MODEL) ** -0.5

f32 = mybir.dt.float32
f32r = mybir.dt.float32r
f16 = mybir.dt.float16
AF = mybir.ActivationFunctionType

MMD = f16              # matmul operand dtype


def build_nc():
    # Bacc: its compile() pass splits multi-sem waits into EventSemaphores
    # (walrus allows only ONE sync wait per engine instruction).
    nc = bacc.Bacc()
    qT = nc.declare_dram_parameter("qT", [D_MODEL, LQ], MMD, isOutput=False)
    kT = nc.declare_dram_parameter("kT", [D_MODEL, LK], MMD, isOutput=False)
    wqT = nc.declare_dram_parameter("wqT", [D_MODEL, D_MODEL], MMD, isOutput=False)
    wkT = nc.declare_dram_parameter("wkT", [D_MODEL, D_MODEL], MMD, isOutput=False)
    wvT = nc.declare_dram_parameter("wvT", [D_MODEL, D_MODEL], MMD, isOutput=False)
    bqT = nc.declare_dram_parameter("bqT", [P, ET], f32, isOutput=False)
    bkT = nc.declare_dram_parameter("bkT", [P, ET], f32, isOutput=False)
    bvB = nc.declare_dram_parameter("bvB", [P, D_MODEL], f32, isOutput=False)
    outT = nc.declare_dram_parameter("outT", [D_MODEL, LQ], f16, isOutput=True)

    qT_r = qT.rearrange("(dt p) i -> p dt i", p=P)
    kT_r = kT.rearrange("(dt p) k -> p dt k", p=P)
    wq_r = wqT.rearrange("(dt p) e -> p dt e", p=P)
    wk_r = wkT.rearrange("(dt p) e -> p dt e", p=P)
    wv_r = wvT.rearrange("(dt p) e -> p dt e", p=P)
    outT_r = outT.rearrange("(et p) i -> p et i", p=P)

    with (
        tile.TileContext(nc) as tc,
        tc.tile_pool(name="big", bufs=1) as big,
        tc.tile_pool(name="work", bufs=3) as work,
        tc.tile_pool(name="mmp", bufs=3, space="PSUM") as mmp,
        tc.tile_pool(name="attp", bufs=4, space="PSUM") as attp,
        tc.tile_pool(name="sump", bufs=1, space="PSUM") as sump,
    ):
        qT_sb = big.tile([P, DT, LQ], MMD, tag="qT")
        kT_sb = big.tile([P, DT, LK], MMD, tag="kT")
        wq_sb = big.tile([P, DT, D_MODEL], MMD, tag="wq")
        wk_sb = big.tile([P, DT, D_MODEL], MMD, tag="wk")
        wv_sb = big.tile([P, DT, D_MODEL], MMD, tag="wv")
        bq_sb = big.tile([P, ET], f32, tag="bq")
        bk_sb = big.tile([P, ET], f32, tag="bk")
        bv_sb = big.tile([P, D_MODEL], f32, tag="bv")
        QT_sb = big.tile([P, ET, LQ], MMD, tag="QT")
        KT_sb = big.tile([P, ET, LK], MMD, tag="KT")
        V_sb = big.tile([P, NKT, D_MODEL], MMD, tag="V")
        out_sb = big.tile([P, ET, LQ], f16, tag="out")
        ones32_sb = big.tile([P, P], f32r, tag="ones32")

        # all-ones stationary tile generated on-device (saves a DMA slot);
        # memset through an f32 view (f32r has no memset value type)
        nc.vector.memset(ones32_sb[:].bitcast(f32), 1.0)

        # ---- input DMAs, ordered so the first matmuls' deps land first;
        # qT whole (2KB lines) / kT in halves (2KB lines) for DMA efficiency
        nc.sync.dma_start(wq_sb[:], wq_r)
        nc.sync.dma_start(bq_sb[:], bqT[:])
        nc.sync.dma_start(qT_sb[:], qT_r)
        nc.sync.dma_start(wk_sb[:], wk_r)
        nc.sync.dma_start(bk_sb[:], bkT[:])
        nc.sync.dma_start(kT_sb[:, :, 0:LK // 2], kT_r[:, :, 0:LK // 2])
        nc.sync.dma_start(wv_sb[:], wv_r)
        nc.sync.dma_start(bv_sb[:], bvB[:])
        nc.sync.dma_start(kT_sb[:, :, LK // 2:], kT_r[:, :, LK // 2:])

        # ---- projections, interleaved by DMA-chunk arrival so the PE
        # fills kT-transfer wait time with already-available work ----
        def q_proj(ic):
            isl = slice(ic * F, (ic + 1) * F)
            for et in range(ET):
                ps = mmp.tile([P, F], f32, tag="mm", name=f"ps_q{ic}{et}")
                for d in range(DT):
                    nc.tensor.matmul(
                        ps[:],
                        wq_sb[:, d, et * P:(et + 1) * P],
                        qT_sb[:, d, isl],
                        start=(d == 0),
                        stop=(d == DT - 1),
                    )
                nc.scalar.activation(
                    QT_sb[:, et, isl], ps[:], AF.Identity, bias=bq_sb[:, et:et + 1]
                )

        def k_proj(kc):
            ksl = slice(kc * F, (kc + 1) * F)
            for et in range(ET):
                ps = mmp.tile([P, F], f32, tag="mm", name=f"ps_k{kc}{et}")
                for d in range(DT):
                    nc.tensor.matmul(
                        ps[:],
                        wk_sb[:, d, et * P:(et + 1) * P],
                        kT_sb[:, d, ksl],
                        start=(d == 0),
                        stop=(d == DT - 1),
                    )
                nc.scalar.activation(
                    KT_sb[:, et, ksl], ps[:], AF.Identity, bias=bk_sb[:, et:et + 1]
                )

        def v_proj(kc):
            for kt in range(4 * kc, 4 * kc + 4):
                ps = mmp.tile([P, F], f32, tag="mm", name=f"ps_v{kt}")
                for d in range(DT):
                    nc.tensor.matmul(
                        ps[:],
                        kT_sb[:, d, kt * P:(kt + 1) * P],
                        wv_sb[:, d, :],
                        start=(d == 0),
                        stop=(d == DT - 1),
                    )
                nc.vector.tensor_add(V_sb[:, kt, :], ps[:], bv_sb[:])

        q_proj(0)
        q_proj(1)
        k_proj(0)
        k_proj(1)
        v_proj(0)
        v_proj(1)
        k_proj(2)
        v_proj(2)
        k_proj(3)
        v_proj(3)

        # ---- attention ----
        for ic in range(NIC):
            isl = slice(ic * F, (ic + 1) * F)
            att = [
                attp.tile([P, F], f32, tag="att", name=f"att_{ic}_{j}")
                for j in range(ET)
            ]
            sumE = work.tile([P, F], f32r, tag="sumE", name=f"sumE_{ic}")

            def s_tile(kt, isl=isl):
                ps = mmp.tile([P, F], f32, tag="mm")
                for et in range(ET):
                    nc.tensor.matmul(
                        ps[:],
                        KT_sb[:, et, kt * P:(kt + 1) * P],
                        QT_sb[:, et, isl],
                        start=(et == 0),
                        stop=(et == ET - 1),
                    )
                return ps

            # software-pipelined, depth 2: S(kt+1), S(kt+2) in flight while
            # exp(kt) runs on ScalarE (covers the ic-transition stall too)
            s_q = [s_tile(0), s_tile(1)]
            for kt in range(NKT):
                if kt + 2 < NKT:
                    s_q.append(s_tile(kt + 2))
                E = work.tile([P, F], MMD, tag="E")
                nc.scalar.activation(E[:], s_q.pop(0)[:], AF.Exp, scale=SCALE)
                # row-sum accumulate on DVE (replaces 16 PE ones-matmuls)
                if kt == 0:
                    nc.vector.tensor_copy(sumE[:], E[:])
                else:
                    nc.vector.tensor_add(sumE[:], sumE[:], E[:])
                for et in range(ET):
                    nc.tensor.matmul(
                        att[et][:],
                        V_sb[:, kt, et * P:(et + 1) * P],
                        E[:],
                        start=(kt == 0),
                        stop=(kt == NKT - 1),
                    )

            # broadcast row-sums to all partitions with one PE pass, then
            # fast approx reciprocal (~18 bits) on DVE
            sum_ps = sump.tile([P, F], f32, tag="sum")
            nc.tensor.matmul(sum_ps[:], ones32_sb[:], sumE[:], start=True, stop=True)
            recip = work.tile([P, F], f32, tag="recip")
            nc.vector.reciprocal_approx_fast(recip[:], sum_ps[:])
            for eh in range(2):  # two half-DMAs so transfer overlaps the muls
                for et in (2 * eh, 2 * eh + 1):
                    nc.vector.tensor_mul(out_sb[:, et, isl], att[et][:], recip[:])
                nc.sync.dma_start(
                    outT_r[:, 2 * eh:2 * eh + 2, isl],
                    out_sb[:, 2 * eh:2 * eh + 2, isl],
                )

    nc.finalize()
    return nc


_NC_CACHE = None


def _get_nc():
    global _NC_CACHE
    if _NC_CACHE is None:
        _NC_CACHE = build_nc()
    return _NC_CACHE


def _prep_in_maps(query, key, Wq, bq, Wk, bk, Wv, bv):
    c = np.ascontiguousarray
    h = np.float16
    shared = {
        "wqT": c(Wq.T.astype(h)),
        "wkT": c(Wk.T.astype(h)),
        "wvT": c(Wv.T.astype(h)),
        "bqT": c(bq.reshape(ET, P).T),
        "bkT": c(bk.reshape(ET, P).T),
        "bvB": c(np.broadcast_to(bv, (P, D_MODEL))),
    }
    return [
        {"qT": c(query[b].T.astype(h)), "kT": c(key[b].T.astype(h)), **shared}
        for b in range(N_CORES)
    ]


def kernel(**inputs):
    query = np.asarray(inputs["query"], np.float32)
    key = np.asarray(inputs["key"], np.float32)
    Wq = np.asarray(inputs["Wq"], np.float32)
    bq = np.asarray(inputs["bq"], np.float32)
    Wk = np.asarray(inputs["Wk"], np.float32)
    bk = np.asarray(inputs["bk"], np.float32)
    Wv = np.asarray(inputs["Wv"], np.float32)
    bv = np.asarray(inputs["bv"], np.float32)

    in_maps = _prep_in_maps(query, key, Wq, bq, Wk, bk, Wv, bv)
    res = run_bass_kernel_spmd(_get_nc(), in_maps, list(range(N_CORES)))
    global LAST_RES
    LAST_RES = res
    out = np.stack(
        [res.results[b]["outT"].astype(np.float32).T for b in range(N_CORES)]
    )
    return np.ascontiguousarray(out)


LAST_RES = None


# revision 8
# speedup vs baseline: 1.2331x; 1.0223x over previous
"""Trainium2 Bass kernel: CrossAttentionBlock, data-parallel over batch on 8 NeuronCores.

Per-core computation (one batch element b):
    Q = query[b] @ Wq.T + bq          [1024, 512]
    K = key[b]   @ Wk.T + bk          [2048, 512]
    V = key[b]   @ Wv.T + bv          [2048, 512]
    S = Q @ K.T / sqrt(512)           [1024, 2048]
    out = softmax(S, axis=-1) @ V     [1024, 512]

Device-side layout ("transposed world": the TensorEngine contraction dim is
always on SBUF partitions, no on-device transposes needed):
  - host passes query[b].T ("qT" [512,1024]), key[b].T ("kT" [512,2048]) and
    transposed weights WqT/WkT/WvT ([d_in, d_out]), all fp16 (same PE rate as
    fp32r, half the DMA/SBUF traffic; end-to-end rel err ~1e-3 vs 2e-2 budget).
  - Q^T[e,i]  = sum_d WqT[d,e] qT[d,i]     (PE, accumulate over 4 d-subtiles)
  - K^T[e,k]  = sum_d WkT[d,e] kT[d,k]
  - V[k,e]    = sum_d kT[d,k] WvT[d,e]
  - S^T[k,i]  = sum_e KT[e,k] QT[e,i]      (per 128-row k-tile, 512-col i-chunk)
  - E = exp(S^T * scale)                   (ScalarE; no max-subtraction needed:
                                            E max ~4e3 fits fp16 range)
  - sumE[p,i] += E[p,i] for k-tiles 0..14  (VectorE accumulate, fp32)
  - sum_ps = ones.T @ sumE + ones.T @ E15  (two PE passes issued before the
                                            last att matmuls so the reciprocal
                                            overlaps them)
  - att^T[e,i] = sum_k V[k,e] E[k,i]       (PE, accumulated over k-tiles)
  - out^T = att^T * recip(sumexp)          (DVE reciprocal_approx_fast;
                                            muls split DVE/GpSimd)
Host transposes out^T back to [1024, 512] per batch element.
"""

import numpy as np

import concourse.bass as bass
import concourse.mybir as mybir
import concourse.tile as tile
from concourse import bacc
from concourse.bass_utils import run_bass_kernel_spmd

P = 128
D_MODEL = 512
DT = D_MODEL // P      # contraction subtiles (4)
ET = D_MODEL // P      # model-dim output tiles (4)
LQ = 1024
LK = 2048
NKT = LK // P          # key tiles (16)
F = 512                # matmul free dim / query-chunk width
NIC = LQ // F          # query chunks (2)
NKC = LK // F          # key chunks for K^T projection (4)
N_CORES = 8
SCALE = float(D_MODEL) ** -0.5

f32 = mybir.dt.float32
f32r = mybir.dt.float32r
f16 = mybir.dt.float16
AF = mybir.ActivationFunctionType

MMD = f16              # matmul operand dtype


def build_nc():
    # Bacc: its compile() pass splits multi-sem waits into EventSemaphores
    # (walrus allows only ONE sync wait per engine instruction).
    nc = bacc.Bacc()
    qT = nc.declare_dram_parameter("qT", [D_MODEL, LQ], MMD, isOutput=False)
    kT = nc.declare_dram_parameter("kT", [D_MODEL, LK], MMD, isOutput=False)
    wqT = nc.declare_dram_parameter("wqT", [D_MODEL, D_MODEL], MMD, isOutput=False)
    wkT = nc.declare_dram_parameter("wkT", [D_MODEL, D_MODEL], MMD, isOutput=False)
    wvT = nc.declare_dram_parameter("wvT", [D_MODEL, D_MODEL], MMD, isOutput=False)
    bqT = nc.declare_dram_parameter("bqT", [P, ET], f32, isOutput=False)
    bkT = nc.declare_dram_parameter("bkT", [P, ET], f32, isOutput=False)
    bvB = nc.declare_dram_parameter("bvB", [P, D_MODEL], f16, isOutput=False)
    outT = nc.declare_dram_parameter("outT", [D_MODEL, LQ], f16, isOutput=True)

    qT_r = qT.rearrange("(dt p) i -> p dt i", p=P)
    kT_r = kT.rearrange("(dt p) k -> p dt k", p=P)
    wq_r = wqT.rearrange("(dt p) e -> p dt e", p=P)
    wk_r = wkT.rearrange("(dt p) e -> p dt e", p=P)
    wv_r = wvT.rearrange("(dt p) e -> p dt e", p=P)
    outT_r = outT.rearrange("(et p) i -> p et i", p=P)

    with (
        tile.TileContext(nc) as tc,
        tc.tile_pool(name="big", bufs=1) as big,
        tc.tile_pool(name="work", bufs=3) as work,
        tc.tile_pool(name="mmp", bufs=4, space="PSUM") as mmp,
        tc.tile_pool(name="attp", bufs=4, space="PSUM") as attp,
    ):
        qT_sb = big.tile([P, DT, LQ], MMD, tag="qT")
        kT_sb = big.tile([P, DT, LK], MMD, tag="kT")
        wq_sb = big.tile([P, DT, D_MODEL], MMD, tag="wq")
        wk_sb = big.tile([P, DT, D_MODEL], MMD, tag="wk")
        wv_sb = big.tile([P, DT, D_MODEL], MMD, tag="wv")
        bq_sb = big.tile([P, ET], f32, tag="bq")
        bk_sb = big.tile([P, ET], f32, tag="bk")
        bv_sb = big.tile([P, D_MODEL], f16, tag="bv")
        QT_sb = big.tile([P, ET, LQ], MMD, tag="QT")
        KT_sb = big.tile([P, ET, LK], MMD, tag="KT")
        V_sb = big.tile([P, NKT, D_MODEL], MMD, tag="V")
        out_sb = big.tile([P, ET, LQ], f16, tag="out")
        ones32_sb = big.tile([P, P], f32r, tag="ones32")
        ones16_sb = big.tile([P, P], f16, tag="ones16")

        # all-ones stationary tiles generated on-device (saves a DMA slot);
        # f32r memset goes through an f32 view (f32r has no memset value type)
        nc.vector.memset(ones32_sb[:].bitcast(f32), 1.0)
        nc.vector.memset(ones16_sb[:], 1.0)

        # ---- input DMAs, ordered so the first matmuls' deps land first.
        # qT split by d-subtile pairs (2KB lines, and q_proj(0) can start
        # its d0/d1 accumulation before d2/d3 arrive); kT in halves.
        nc.sync.dma_start(wq_sb[:], wq_r)
        nc.sync.dma_start(bq_sb[:], bqT[:])
        nc.sync.dma_start(qT_sb[:, 0:2, :], qT_r[:, 0:2, :])
        nc.sync.dma_start(qT_sb[:, 2:4, :], qT_r[:, 2:4, :])
        nc.sync.dma_start(wk_sb[:], wk_r)
        nc.sync.dma_start(bk_sb[:], bkT[:])
        nc.sync.dma_start(kT_sb[:, :, 0:LK // 2], kT_r[:, :, 0:LK // 2])
        nc.sync.dma_start(wv_sb[:], wv_r)
        nc.sync.dma_start(bv_sb[:], bvB[:])
        nc.sync.dma_start(kT_sb[:, :, LK // 2:], kT_r[:, :, LK // 2:])

        # ---- projections ----
        def q_proj_split(ic):
            # d0/d1 matmuls for all four et tiles first (only needs the first
            # qT half-DMA), then d2/d3 + bias; holds 4 psum banks
            isl = slice(ic * F, (ic + 1) * F)
            pss = [
                mmp.tile([P, F], f32, tag="mm", name=f"ps_q{ic}{et}")
                for et in range(ET)
            ]
            for dh in range(2):
                for et in range(ET):
                    for d in (2 * dh, 2 * dh + 1):
                        nc.tensor.matmul(
                            pss[et][:],
                            wq_sb[:, d, et * P:(et + 1) * P],
                            qT_sb[:, d, isl],
                            start=(d == 0),
                            stop=(d == DT - 1),
                        )
                    if dh == 1:
                        nc.scalar.activation(
                            QT_sb[:, et, isl], pss[et][:], AF.Identity,
                            bias=bq_sb[:, et:et + 1],
                        )

        def q_proj(ic):
            isl = slice(ic * F, (ic + 1) * F)
            for et in range(ET):
                ps = mmp.tile([P, F], f32, tag="mm", name=f"ps_q{ic}{et}")
                for d in range(DT):
                    nc.tensor.matmul(
                        ps[:],
                        wq_sb[:, d, et * P:(et + 1) * P],
                        qT_sb[:, d, isl],
                        start=(d == 0),
                        stop=(d == DT - 1),
                    )
                nc.scalar.activation(
                    QT_sb[:, et, isl], ps[:], AF.Identity, bias=bq_sb[:, et:et + 1]
                )

        def k_proj(kc):
            ksl = slice(kc * F, (kc + 1) * F)
            for et in range(ET):
                ps = mmp.tile([P, F], f32, tag="mm", name=f"ps_k{kc}{et}")
                for d in range(DT):
                    nc.tensor.matmul(
                        ps[:],
                        wk_sb[:, d, et * P:(et + 1) * P],
                        kT_sb[:, d, ksl],
                        start=(d == 0),
                        stop=(d == DT - 1),
                    )
                nc.scalar.activation(
                    KT_sb[:, et, ksl], ps[:], AF.Identity, bias=bk_sb[:, et:et + 1]
                )

        def v_proj(kc):
            for kt in range(4 * kc, 4 * kc + 4):
                ps = mmp.tile([P, F], f32, tag="mm", name=f"ps_v{kt}")
                for d in range(DT):
                    nc.tensor.matmul(
                        ps[:],
                        kT_sb[:, d, kt * P:(kt + 1) * P],
                        wv_sb[:, d, :],
                        start=(d == 0),
                        stop=(d == DT - 1),
                    )
                nc.vector.tensor_add(V_sb[:, kt, :], ps[:], bv_sb[:])

        q_proj_split(0)
        q_proj(1)
        k_proj(0)
        k_proj(1)
        v_proj(0)
        v_proj(1)
        k_proj(2)
        v_proj(2)
        k_proj(3)
        v_proj(3)

        # ---- attention ----
        for ic in range(NIC):
            isl = slice(ic * F, (ic + 1) * F)
            att = [
                attp.tile([P, F], f32, tag="att", name=f"att_{ic}_{j}")
                for j in range(ET)
            ]
            sumE = work.tile([P, F], f32r, tag="sumE", name=f"sumE_{ic}")

            def s_tile(kt, isl=isl):
                ps = mmp.tile([P, F], f32, tag="mm")
                for et in range(ET):
                    nc.tensor.matmul(
                        ps[:],
                        KT_sb[:, et, kt * P:(kt + 1) * P],
                        QT_sb[:, et, isl],
                        start=(et == 0),
                        stop=(et == ET - 1),
                    )
                return ps

            # software-pipelined, depth 2: S(kt+1), S(kt+2) in flight while
            # exp(kt) runs on ScalarE (covers the ic-transition stall too)
            s_q = [s_tile(0), s_tile(1)]
            sum_ps = None
            recip = work.tile([P, F], f32, tag="recip", name=f"recip_{ic}")
            for kt in range(NKT):
                if kt + 2 < NKT:
                    s_q.append(s_tile(kt + 2))
                E = work.tile([P, F], MMD, tag="E")
                nc.scalar.activation(E[:], s_q.pop(0)[:], AF.Exp, scale=SCALE)
                if kt == 0:
                    # row-sum accumulate on DVE (replaces 16 PE ones-matmuls)
                    nc.vector.tensor_copy(sumE[:], E[:])
                elif kt < NKT - 1:
                    nc.vector.tensor_add(sumE[:], sumE[:], E[:])
                else:
                    # total = ones.T@sumE(0..14) + ones.T@E15, issued BEFORE
                    # the final att matmuls so recip overlaps them on DVE
                    sum_ps = mmp.tile([P, F], f32, tag="mm", name=f"sum_{ic}")
                    nc.tensor.matmul(
                        sum_ps[:], ones32_sb[:], sumE[:], start=True, stop=False
                    )
                    nc.tensor.matmul(
                        sum_ps[:], ones16_sb[:], E[:], start=False, stop=True
                    )
                    nc.vector.reciprocal_approx_fast(recip[:], sum_ps[:])
                for et in range(ET):
                    nc.tensor.matmul(
                        att[et][:],
                        V_sb[:, kt, et * P:(et + 1) * P],
                        E[:],
                        start=(kt == 0),
                        stop=(kt == NKT - 1),
                    )

            # final normalize (GpSimd can't read PSUM, so all muls on DVE),
            # two half-DMAs so transfer overlaps the remaining muls
            for eh in range(2):
                for et in (2 * eh, 2 * eh + 1):
                    nc.vector.tensor_mul(out_sb[:, et, isl], att[et][:], recip[:])
                nc.sync.dma_start(
                    outT_r[:, 2 * eh:2 * eh + 2, isl],
                    out_sb[:, 2 * eh:2 * eh + 2, isl],
                )

    nc.finalize()
    return nc


_NC_CACHE = None


def _get_nc():
    global _NC_CACHE
    if _NC_CACHE is None:
        _NC_CACHE = build_nc()
    return _NC_CACHE


def _prep_in_maps(query, key, Wq, bq, Wk, bk, Wv, bv):
    c = np.ascontiguousarray
    h = np.float16
    shared = {
        "wqT": c(Wq.T.astype(h)),
        "wkT": c(Wk.T.astype(h)),
        "wvT": c(Wv.T.astype(h)),
        "bqT": c(bq.reshape(ET, P).T),
        "bkT": c(bk.reshape(ET, P).T),
        "bvB": c(np.broadcast_to(bv, (P, D_MODEL)).astype(h)),
    }
    return [
        {"qT": c(query[b].T.astype(h)), "kT": c(key[b].T.astype(h)), **shared}
        for b in range(N_CORES)
    ]


def kernel(**inputs):
    query = np.asarray(inputs["query"], np.float32)
    key = np.asarray(inputs["key"], np.float32)
    Wq = np.asarray(inputs["Wq"], np.float32)
    bq = np.asarray(inputs["bq"], np.float32)
    Wk = np.asarray(inputs["Wk"], np.float32)
    bk = np.asarray(inputs["bk"], np.float32)
    Wv = np.asarray(inputs["Wv"], np.float32)
    bv = np.asarray(inputs["bv"], np.float32)

    in_maps = _prep_in_maps(query, key, Wq, bq, Wk, bk, Wv, bv)
    res = run_bass_kernel_spmd(_get_nc(), in_maps, list(range(N_CORES)))
    global LAST_RES
    LAST_RES = res
    out = np.stack(
        [res.results[b]["outT"].astype(np.float32).T for b in range(N_CORES)]
    )
    return np.ascontiguousarray(out)


LAST_RES = None


# revision 9
# speedup vs baseline: 1.4100x; 1.1434x over previous
"""Trainium2 Bass kernel: CrossAttentionBlock, data-parallel over batch on 8 NeuronCores.

Per-core computation (one batch element b):
    Q = query[b] @ Wq.T + bq          [1024, 512]
    K = key[b]   @ Wk.T + bk          [2048, 512]
    V = key[b]   @ Wv.T + bv          [2048, 512]
    S = Q @ K.T / sqrt(512)           [1024, 2048]
    out = softmax(S, axis=-1) @ V     [1024, 512]

Key algebraic fusion (saves the whole K projection, 64 matmuls/core):
    S_ij = q_i (Wq^T Wk) k_j + u_i + w_j + const,  u_i = q_i Wq^T bk
    Softmax over j is invariant to u_i and const, so with host-precomputed
    M = Wq^T Wk and w = (key @ Wk^T bq) * scale:
        softmax(S/sqrt(d)) == softmax((q M k^T)*scale + w)
    w folds into the exp as ScalarE's free per-partition bias.

Device-side layout ("transposed world": the TensorEngine contraction dim is
always on SBUF partitions, no on-device transposes needed). All matmul
operands fp16 (same PE rate as fp32r, half the DMA/SBUF traffic; end-to-end
rel err ~8e-4 vs 2e-2 budget):
  - T^T[e,i]  = sum_d M[d,e] qT[d,i]       (PE, accumulate over 4 d-subtiles)
  - V[k,e]    = sum_d kT[d,k] WvT[d,e]
  - S^T[k,i]  = sum_d kT[d,k] T^T[d,i]     (per 128-row k-tile, 512-col i-chunk)
  - E = exp(S^T * scale + w[k])            (ScalarE; no max-subtraction needed:
                                            E max ~4e3 fits fp16 range)
  - sumE[p,i] += E[p,i] for k-tiles 0..14  (VectorE accumulate, fp32)
  - sum_ps = ones.T @ sumE + ones.T @ E15  (two PE passes issued before the
                                            last att matmuls so the reciprocal
                                            overlaps them)
  - att^T[e,i] = sum_k V[k,e] E[k,i]       (PE, accumulated over k-tiles)
  - out^T = att^T * recip(sumexp)          (DVE reciprocal_approx_fast + muls)
Host transposes out^T back to [1024, 512] per batch element.
"""

import numpy as np

import concourse.bass as bass
import concourse.mybir as mybir
import concourse.tile as tile
from concourse import bacc
from concourse.bass_utils import run_bass_kernel_spmd

P = 128
D_MODEL = 512
DT = D_MODEL // P      # contraction subtiles (4)
ET = D_MODEL // P      # model-dim output tiles (4)
LQ = 1024
LK = 2048
NKT = LK // P          # key tiles (16)
F = 512                # matmul free dim / query-chunk width
NIC = LQ // F          # query chunks (2)
N_CORES = 8
SCALE = float(D_MODEL) ** -0.5

f32 = mybir.dt.float32
f32r = mybir.dt.float32r
f16 = mybir.dt.float16
AF = mybir.ActivationFunctionType

MMD = f16              # matmul operand dtype


def build_nc():
    # Bacc: its compile() pass splits multi-sem waits into EventSemaphores
    # (walrus allows only ONE sync wait per engine instruction).
    nc = bacc.Bacc()
    qT = nc.declare_dram_parameter("qT", [D_MODEL, LQ], MMD, isOutput=False)
    kT = nc.declare_dram_parameter("kT", [D_MODEL, LK], MMD, isOutput=False)
    m = nc.declare_dram_parameter("m", [D_MODEL, D_MODEL], MMD, isOutput=False)
    wvT = nc.declare_dram_parameter("wvT", [D_MODEL, D_MODEL], MMD, isOutput=False)
    wbias = nc.declare_dram_parameter("wbias", [P, NKT], f32, isOutput=False)
    bvB = nc.declare_dram_parameter("bvB", [P, D_MODEL], f16, isOutput=False)
    outT = nc.declare_dram_parameter("outT", [D_MODEL, LQ], f16, isOutput=True)

    qT_r = qT.rearrange("(dt p) i -> p dt i", p=P)
    kT_r = kT.rearrange("(dt p) k -> p dt k", p=P)
    m_r = m.rearrange("(dt p) e -> p dt e", p=P)
    wv_r = wvT.rearrange("(dt p) e -> p dt e", p=P)
    outT_r = outT.rearrange("(et p) i -> p et i", p=P)

    with (
        tile.TileContext(nc) as tc,
        tc.tile_pool(name="big", bufs=1) as big,
        tc.tile_pool(name="work", bufs=3) as work,
        tc.tile_pool(name="mmp", bufs=4, space="PSUM") as mmp,
        tc.tile_pool(name="attp", bufs=4, space="PSUM") as attp,
    ):
        qT_sb = big.tile([P, DT, LQ], MMD, tag="qT")
        kT_sb = big.tile([P, DT, LK], MMD, tag="kT")
        m_sb = big.tile([P, DT, D_MODEL], MMD, tag="m")
        wv_sb = big.tile([P, DT, D_MODEL], MMD, tag="wv")
        w_sb = big.tile([P, NKT], f32, tag="wbias")
        bv_sb = big.tile([P, D_MODEL], f16, tag="bv")
        T_sb = big.tile([P, DT, LQ], MMD, tag="T")
        V_sb = big.tile([P, NKT, D_MODEL], MMD, tag="V")
        out_sb = big.tile([P, ET, LQ], f16, tag="out")
        ones32_sb = big.tile([P, P], f32r, tag="ones32")
        ones16_sb = big.tile([P, P], f16, tag="ones16")

        # all-ones stationary tiles generated on-device (saves a DMA slot);
        # f32r memset goes through an f32 view (f32r has no memset value type)
        nc.vector.memset(ones32_sb[:].bitcast(f32), 1.0)
        nc.vector.memset(ones16_sb[:], 1.0)

        # ---- input DMAs, ordered so the first matmuls' deps land first.
        # m and qT split by d-subtile pairs (2KB lines for qT/kT, and the
        # T-projection can start its d0/d1 accumulation before d2/d3 arrive)
        nc.sync.dma_start(m_sb[:, 0:2, :], m_r[:, 0:2, :])
        nc.sync.dma_start(qT_sb[:, 0:2, :], qT_r[:, 0:2, :])
        nc.sync.dma_start(m_sb[:, 2:4, :], m_r[:, 2:4, :])
        nc.sync.dma_start(qT_sb[:, 2:4, :], qT_r[:, 2:4, :])
        nc.sync.dma_start(w_sb[:], wbias[:])
        nc.sync.dma_start(kT_sb[:, :, 0:LK // 2], kT_r[:, :, 0:LK // 2])
        nc.sync.dma_start(wv_sb[:], wv_r)
        nc.sync.dma_start(bv_sb[:], bvB[:])
        nc.sync.dma_start(kT_sb[:, :, LK // 2:], kT_r[:, :, LK // 2:])

        # ---- T = q @ M projection ----
        def t_proj_split(ic):
            # d0/d1 matmuls for all four et tiles first (only needs the first
            # m/qT half-DMAs), then d2/d3; holds 4 psum banks
            isl = slice(ic * F, (ic + 1) * F)
            pss = [
                mmp.tile([P, F], f32, tag="mm", name=f"ps_t{ic}{et}")
                for et in range(ET)
            ]
            for dh in range(2):
                for et in range(ET):
                    for d in (2 * dh, 2 * dh + 1):
                        nc.tensor.matmul(
                            pss[et][:],
                            m_sb[:, d, et * P:(et + 1) * P],
                            qT_sb[:, d, isl],
                            start=(d == 0),
                            stop=(d == DT - 1),
                        )
                    if dh == 1:
                        nc.scalar.activation(
                            T_sb[:, et, isl], pss[et][:], AF.Identity
                        )

        def t_proj(ic):
            isl = slice(ic * F, (ic + 1) * F)
            for et in range(ET):
                ps = mmp.tile([P, F], f32, tag="mm", name=f"ps_t{ic}{et}")
                for d in range(DT):
                    nc.tensor.matmul(
                        ps[:],
                        m_sb[:, d, et * P:(et + 1) * P],
                        qT_sb[:, d, isl],
                        start=(d == 0),
                        stop=(d == DT - 1),
                    )
                nc.scalar.activation(T_sb[:, et, isl], ps[:], AF.Identity)

        def v_proj(kc):
            for kt in range(4 * kc, 4 * kc + 4):
                ps = mmp.tile([P, F], f32, tag="mm", name=f"ps_v{kt}")
                for d in range(DT):
                    nc.tensor.matmul(
                        ps[:],
                        kT_sb[:, d, kt * P:(kt + 1) * P],
                        wv_sb[:, d, :],
                        start=(d == 0),
                        stop=(d == DT - 1),
                    )
                nc.vector.tensor_add(V_sb[:, kt, :], ps[:], bv_sb[:])

        t_proj_split(0)
        t_proj(1)
        v_proj(0)
        v_proj(1)
        v_proj(2)
        v_proj(3)

        # ---- attention ----
        for ic in range(NIC):
            isl = slice(ic * F, (ic + 1) * F)
            att = [
                attp.tile([P, F], f32, tag="att", name=f"att_{ic}_{j}")
                for j in range(ET)
            ]
            sumE = work.tile([P, F], f32r, tag="sumE", name=f"sumE_{ic}")

            def s_tile(kt, isl=isl):
                ps = mmp.tile([P, F], f32, tag="mm")
                for d in range(DT):
                    nc.tensor.matmul(
                        ps[:],
                        kT_sb[:, d, kt * P:(kt + 1) * P],
                        T_sb[:, d, isl],
                        start=(d == 0),
                        stop=(d == DT - 1),
                    )
                return ps

            # software-pipelined, depth 2: S(kt+1), S(kt+2) in flight while
            # exp(kt) runs on ScalarE (covers the ic-transition stall too)
            s_q = [s_tile(0), s_tile(1)]
            recip = work.tile([P, F], f32, tag="recip", name=f"recip_{ic}")
            for kt in range(NKT):
                if kt + 2 < NKT:
                    s_q.append(s_tile(kt + 2))
                E = work.tile([P, F], MMD, tag="E")
                nc.scalar.activation(
                    E[:], s_q.pop(0)[:], AF.Exp, scale=SCALE,
                    bias=w_sb[:, kt:kt + 1],
                )
                if kt == 0:
                    # row-sum accumulate on DVE (replaces 16 PE ones-matmuls)
                    nc.vector.tensor_copy(sumE[:], E[:])
                elif kt < NKT - 1:
                    nc.vector.tensor_add(sumE[:], sumE[:], E[:])
                else:
                    # total = ones.T@sumE(0..14) + ones.T@E15, issued BEFORE
                    # the final att matmuls so recip overlaps them on DVE
                    sum_ps = mmp.tile([P, F], f32, tag="mm", name=f"sum_{ic}")
                    nc.tensor.matmul(
                        sum_ps[:], ones32_sb[:], sumE[:], start=True, stop=False
                    )
                    nc.tensor.matmul(
                        sum_ps[:], ones16_sb[:], E[:], start=False, stop=True
                    )
                    nc.vector.reciprocal_approx_fast(recip[:], sum_ps[:])
                for et in range(ET):
                    nc.tensor.matmul(
                        att[et][:],
                        V_sb[:, kt, et * P:(et + 1) * P],
                        E[:],
                        start=(kt == 0),
                        stop=(kt == NKT - 1),
                    )

            # final normalize on DVE, two half-DMAs so transfer overlaps muls
            for eh in range(2):
                for et in (2 * eh, 2 * eh + 1):
                    nc.vector.tensor_mul(out_sb[:, et, isl], att[et][:], recip[:])
                nc.sync.dma_start(
                    outT_r[:, 2 * eh:2 * eh + 2, isl],
                    out_sb[:, 2 * eh:2 * eh + 2, isl],
                )

    nc.finalize()
    return nc


_NC_CACHE = None


def _get_nc():
    global _NC_CACHE
    if _NC_CACHE is None:
        _NC_CACHE = build_nc()
    return _NC_CACHE


def _prep_in_maps(query, key, Wq, bq, Wk, bk, Wv, bv):
    c = np.ascontiguousarray
    h = np.float16
    M = (Wq.T @ Wk).astype(h)          # S = q M k^T (+ bias terms, see header)
    a = Wk.T @ bq
    shared = {
        "m": c(M),
        "wvT": c(Wv.T.astype(h)),
        "bvB": c(np.broadcast_to(bv, (P, D_MODEL)).astype(h)),
    }
    maps = []
    for b in range(N_CORES):
        w = (key[b] @ a) * SCALE       # [LK] per-key softmax bias
        maps.append({
            "qT": c(query[b].T.astype(h)),
            "kT": c(key[b].T.astype(h)),
            "wbias": c(w.reshape(NKT, P).T.astype(np.float32)),
            **shared,
        })
    return maps


def kernel(**inputs):
    query = np.asarray(inputs["query"], np.float32)
    key = np.asarray(inputs["key"], np.float32)
    Wq = np.asarray(inputs["Wq"], np.float32)
    bq = np.asarray(inputs["bq"], np.float32)
    Wk = np.asarray(inputs["Wk"], np.float32)
    bk = np.asarray(inputs["bk"], np.float32)
    Wv = np.asarray(inputs["Wv"], np.float32)
    bv = np.asarray(inputs["bv"], np.float32)

    in_maps = _prep_in_maps(query, key, Wq, bq, Wk, bk, Wv, bv)
    res = run_bass_kernel_spmd(_get_nc(), in_maps, list(range(N_CORES)))
    global LAST_RES
    LAST_RES = res
    out = np.stack(
        [res.results[b]["outT"].astype(np.float32).T for b in range(N_CORES)]
    )
    return np.ascontiguousarray(out)


LAST_RES = None
